# revision 28
# baseline (speedup 1.0000x reference)
"""Trainium2 Bass kernel for nn_Decoder_4561255269164 (retrieval_knn).

Math: the reference's top-K(8) KNN collapses to min-reductions:
  - backward: weight w=1/sqrt(d) is nonzero only where d equals the row min
    (over kept candidates), so the scatter-add num/den equals
    E_b^T @ [w*rgb, w] with E_b[i,j] = (d2[i,j] == rowmin_i).
  - forward: only the column argmin rows of d2 matter; sumf/cntf =
    E_f^T @ [rgb, 1] with E_f[i,j] = (d2[i,j] <= colmin_j).
  - exact-match (d==0) rows need no separate path: the ungated weight
    1/sqrt(max(m,1e-30)) = 1e15 dominates the weighted mean identically.

Key reductions:
  * every path above only involves KEPT candidates (non-kept are masked to
    BIG in the KNN and excluded from the loss), and #kept <= 8192 by
    construction, so the host compacts candidates to LK=8192 columns (pad
    columns get b2=BIG so they never win a min, keepf=0 so they never reach
    the loss).
  * the bf16 d2 matrix (10 tiles x [128, 8192] = 160KB/partition) is
    PERSISTED in SBUF, so pass B needs no d2 recompute and no second round
    of PSUM->SBUF relu copies (the scalar engine was half the old cost).
  * the eb/ef indicator matmuls (M=8/4, K=128) are column-tiled 4-wide via
    tile_position=(0,32q) into one [128,512] PSUM bank, quadrupling PE
    throughput; lhsT is zero-padded to 32 columns so every PSUM partition is
    written (whole-tile drains read no garbage).

Sharding: targets (N) split across cores (padded to NT*128 rows each).
Pass A computes d2 tile-by-tile in bf16 (tile_position-packed contract-5
matmuls, 4x concurrent; the c5 rhs is pre-grouped on the host into the 4
tile-position partition homes at quarter width).  Row mins use TT-min
accumulation at DVE 2x + folds + one narrow 1x reduce per target tile;
colpart accumulates elementwise column mins across target tiles.  Column
partials collapse across partitions with PE transposes + DVE reduces, then
AllReduce(min) across cores.
Pass B walks 4 column chunks: all eb (backward) chunks run first, hiding the
colmin AllReduce(min); each ef (forward) chunk then compares the persisted
d2 against the broadcast global colmin, and its [8, lch] scatter partials
AllReduce(add) per chunk so collectives overlap the next chunk's compute.
The finalize is chunked the same way and the BCE term (over the FULL 16384
pred_F) is computed between the passes.
"""

import numpy as np

import concourse.bass as bass
import concourse.bacc as bacc
import concourse.bass_isa as bass_isa
import concourse.mybir as mybir
import concourse.tile as tile
from concourse import library_config
from concourse.bass_utils import run_bass_kernel_spmd

F32 = mybir.dt.float32
BF16 = mybir.dt.bfloat16
AX = mybir.AxisListType
ALU = mybir.AluOpType
ACTF = mybir.ActivationFunctionType

# geometry
NCORES = 8
LBCE = 16384       # full candidate count (BCE over pred_F / keep_target)
LK = 8192          # compacted kept candidates (#kept <= 8192 always)
N = 10000          # targets
NT = 10            # i-tiles of 128 per core (pad 1250 -> 1280)
BIG = np.float32(1e30)

AT_W = 2048        # pass A window (NGA=4 tile-position groups of 512)
NGA = AT_W // 512
NDCH = 4           # nd AllReduce chunks == pass B column chunks


def _build_nc(reps=1, phases=("A", "C", "W", "B", "FIN")):
    npad = NT * 128

    nc = bacc.Bacc("TRN2", target_bir_lowering=False, debug=False,
                   num_devices=NCORES)

    # c5 pre-grouped by tile-position: [5, g*2048 + jc*512 + u] =
    # c5[:, jc*2048 + g*512 + u]
    c5gd = nc.declare_dram_parameter("c5g", [5, LK], BF16, isOutput=False)
    t5d = nc.declare_dram_parameter("t5", [5, npad], BF16, isOutput=False)
    trgbd = nc.declare_dram_parameter("trgb", [128, NT * 3], F32, isOutput=False)
    rgbpd = nc.declare_dram_parameter("rgbp", [3, LK], F32, isOutput=False)
    keepd = nc.declare_dram_parameter("keepf", [1, LK], F32, isOutput=False)
    predd = nc.declare_dram_parameter("predf", [1, LBCE], F32, isOutput=False)
    ktgtd = nc.declare_dram_parameter("ktgt", [1, LBCE], F32, isOutput=False)
    eyed = nc.declare_dram_parameter("eye128", [128, 128], F32, isOutput=False)
    chaind = nc.declare_dram_parameter("chain", [1, 2], F32, isOutput=False)
    outd = nc.declare_dram_parameter("out", [1, 2], F32, isOutput=True)

    rg = [list(range(NCORES))]
    lch = LK // NDCH         # columns per nd/pass-B chunk (2048)
    lpb = LBCE // 128        # BCE plane free width
    lpf = LK // 128          # finalize plane free width
    nct = LK // 128          # colmin transpose tiles
    nat = LK // AT_W         # pass A windows
    nq = lch // 512          # matmul column groups per chunk

    with tile.TileContext(nc) as tc:
        nc.gpsimd.load_library(library_config.mlp)
        for _rep in range(reps):
            with (
                tc.tile_pool(name="persist", bufs=1) as pp,
                tc.tile_pool(name="dram", bufs=1, space="DRAM") as dp,
            ):
                # ---------------- persistent SBUF loads --------------------
                t5b = pp.tile([(NGA - 1) * 32 + 5, npad], BF16, tag="t5b",
                              name="t5b")
                for g in range(NGA):
                    nc.sync.dma_start(t5b[32 * g:32 * g + 5, :], t5d[:, :])
                trgb = pp.tile([128, NT * 3], F32, tag="trgb", name="trgb")
                nc.sync.dma_start(trgb[:], trgbd[:, :])
                eye = pp.tile([128, 128], F32, tag="eye", name="eye")
                nc.sync.dma_start(eye[:], eyed[:, :])
                eyeb = pp.tile([128, 128], BF16, tag="eyeb", name="eyeb")
                nc.vector.tensor_copy(eyeb[:], eye[:])
                m2loc = pp.tile([128, nct], F32, tag="m2loc")  # [p, jt]

                m_all = pp.tile([128, NT], F32, tag="m_all")   # row mins
                m_relu = pp.tile([128, NT], F32, tag="m_relu")
                wb_all = pp.tile([128, NT * 32], BF16, tag="wb_all")
                wf_all = pp.tile([128, NT * 32], BF16, tag="wf_all")
                rowsbce = pp.tile([128, 1], F32, tag="rowsbce")
                kscale = pp.tile([128, NT], F32, tag="kscale")
                kbias = pp.tile([128, NT], F32, tag="kbias")

                # persisted bf16 d2: tile t holds targets t*128..t*128+127
                d2p = [pp.tile([128, LK], BF16, tag=f"d2p{t}",
                               name=f"d2p{t}") for t in range(NT)]

                # collectives proved load-safe only with f32 payloads on
                # whole (unsliced) DRAM tensors
                m2_in = dp.tile([nct, 128], F32, tag="m2_in")
                m2_out = dp.tile([1, LK], F32, tag="m2_out")
                nd_ins = [dp.tile([8, lch], F32, tag=f"nd_in{ch}",
                                  name=f"nd_in{ch}") for ch in range(NDCH)]
                nd_outs = [dp.tile([8, lch], F32, tag=f"nd_out{ch}",
                                   name=f"nd_out{ch}") for ch in range(NDCH)]

                with tc.tile_pool(name="colp", bufs=1) as cpp:
                    colpart = cpp.tile([128, LK], BF16, tag="colpart")

                    if "A" in phases:
                        # --- Pass A: d2 into d2p; row mins + col mins ------
                        # window-major so each window's colmin transposes
                        # (phase C) pipeline behind its col chain instead of
                        # serializing after pass A.  Row mins fold each
                        # window 2048->512 into per-tile accumulators.
                        with (
                            tc.tile_pool(name="a_c5", bufs=1) as ac5,
                            tc.tile_pool(name="a_ps", bufs=3,
                                         space="PSUM") as apsp,
                            tc.tile_pool(name="c_ps", bufs=2,
                                         space="PSUM") as cps,
                            tc.tile_pool(name="a_r", bufs=1) as arp,
                            tc.tile_pool(name="a_ra", bufs=1) as arap,
                        ):
                            c5g = ac5.tile([(NGA - 1) * 32 + 5, LK // NGA],
                                           BF16, tag="c5g", name="c5g")
                            for g in range(NGA):
                                nc.sync.dma_start(
                                    c5g[32 * g:32 * g + 5, :],
                                    c5gd[:, g * (LK // NGA):
                                         (g + 1) * (LK // NGA)])
                            rowacc = [arap.tile([128, 512], BF16,
                                                tag=f"rowacc{t}",
                                                name=f"rowacc{t}")
                                      for t in range(NT)]
                            for jc in range(nat):
                                wsl = slice(jc * AT_W, (jc + 1) * AT_W)
                                for t in range(NT):
                                    dsl = d2p[t][:, wsl]
                                    # two [128,1024] psum tiles per window
                                    # (6 banks at bufs=3, leaving 2 for the
                                    # inline colmin transposes)
                                    for h in range(2):
                                        ps = apsp.tile([128, AT_W // 2], F32,
                                                       tag="aps")
                                        for g in (2 * h, 2 * h + 1):
                                            q0 = (g - 2 * h) * 512
                                            nc.tensor.matmul(
                                                ps[:, q0:q0 + 512],
                                                lhsT=t5b[32 * g:32 * g + 5,
                                                         t * 128:
                                                         (t + 1) * 128],
                                                rhs=c5g[32 * g:32 * g + 5,
                                                        jc * 512:
                                                        (jc + 1) * 512],
                                                start=True, stop=True,
                                                tile_position=(32 * g, 0))
                                        nc.scalar.activation(
                                            d2p[t][:, jc * AT_W +
                                                   h * (AT_W // 2):
                                                   jc * AT_W +
                                                   (h + 1) * (AT_W // 2)],
                                            ps[:], ACTF.Relu)
                                    # row-min fold 2048 -> 512 -> accumulate
                                    f1 = arp.tile([128, AT_W // 2], BF16,
                                                  tag="f1")
                                    nc.vector.tensor_tensor(
                                        f1[:], dsl[:, 0:AT_W // 2],
                                        dsl[:, AT_W // 2:AT_W], op=ALU.min)
                                    if jc == 0:
                                        nc.vector.tensor_tensor(
                                            rowacc[t][:], f1[:, 0:AT_W // 4],
                                            f1[:, AT_W // 4:AT_W // 2],
                                            op=ALU.min)
                                    else:
                                        f2 = arp.tile([128, AT_W // 4], BF16,
                                                      tag="f2")
                                        nc.vector.tensor_tensor(
                                            f2[:], f1[:, 0:AT_W // 4],
                                            f1[:, AT_W // 4:AT_W // 2],
                                            op=ALU.min)
                                        nc.vector.tensor_tensor(
                                            rowacc[t][:], rowacc[t][:], f2[:],
                                            op=ALU.min)
                                    # col-min accumulation across tiles
                                    if t == 1:
                                        nc.vector.tensor_tensor(
                                            colpart[:, wsl], d2p[0][:, wsl],
                                            dsl, op=ALU.min)
                                    elif t > 1:
                                        nc.vector.tensor_tensor(
                                            colpart[:, wsl], colpart[:, wsl],
                                            dsl, op=ALU.min)
                                    # previous window's colmin transposes,
                                    # sprinkled so they never head-of-line
                                    # block the PE behind a pending colchain
                                    if "C" in phases and jc > 0 and t < 8:
                                        for jt in (
                                                (jc - 1) * (AT_W // 128)
                                                + 2 * t,
                                                (jc - 1) * (AT_W // 128)
                                                + 2 * t + 1):
                                            pst = cps.tile([128, 128], BF16,
                                                           tag="pstb")
                                            nc.tensor.transpose(
                                                pst[:],
                                                colpart[:, jt * 128:
                                                        (jt + 1) * 128],
                                                eyeb[:])
                                            nc.vector.tensor_reduce(
                                                m2loc[:, jt:jt + 1], pst[:],
                                                axis=AX.X, op=ALU.min)
                            if "C" in phases:
                                # last window's transposes
                                for jt in range((nat - 1) * (AT_W // 128),
                                                nat * (AT_W // 128)):
                                    pst = cps.tile([128, 128], BF16,
                                                   tag="pstb")
                                    nc.tensor.transpose(
                                        pst[:],
                                        colpart[:, jt * 128:(jt + 1) * 128],
                                        eyeb[:])
                                    nc.vector.tensor_reduce(
                                        m2loc[:, jt:jt + 1], pst[:],
                                        axis=AX.X, op=ALU.min)
                            # final row-min per tile: fold 512->256 + reduce
                            for t in range(NT):
                                nc.vector.tensor_tensor(
                                    rowacc[t][:, 0:256], rowacc[t][:, 0:256],
                                    rowacc[t][:, 256:512], op=ALU.min)
                                nc.vector.tensor_reduce(
                                    m_all[:, t:t + 1], rowacc[t][:, 0:256],
                                    axis=AX.X, op=ALU.min)

                    if "W" in phases:
                        # ---------------- weight tiles ----------------------
                        with tc.tile_pool(name="wsmall", bufs=1) as ws:
                            nc.vector.tensor_scalar(m_relu[:], m_all[:], 0.0,
                                                    None, op0=ALU.max)
                            msafe = ws.tile([128, NT], F32, tag="msafe")
                            nc.vector.tensor_scalar(msafe[:], m_relu[:], 1e-30,
                                                    None, op0=ALU.max)
                            sqm = ws.tile([128, NT], F32, tag="sqm")
                            nc.scalar.activation(sqm[:], msafe[:], ACTF.Sqrt)
                            w0 = ws.tile([128, NT], F32, tag="w0")
                            nc.vector.reciprocal(w0[:], sqm[:])
                            # no m>0 gate: zero-distance rows get w=1e15,
                            # which dominates the num/den weighted mean and
                            # reproduces the exact-match override (no
                            # separate s0/cnt0 scatter rows needed)
                            vv = ws.tile([128, NT], F32, tag="vv")
                            nc.vector.tensor_scalar(vv[:], m_relu[:], 1e29,
                                                    None, op0=ALU.is_lt)
                            wgt = ws.tile([128, NT], F32, tag="wgt")
                            nc.vector.tensor_tensor(wgt[:], w0[:], vv[:],
                                                    op=ALU.mult)

                            # eb hat indicator relu(1 - K*(d2-m)) with
                            # K = 256/m: the hat hits exactly zero at one
                            # bf16 ulp above m, so on bf16 d2 it equals
                            # is_equal -- but runs on the idle ACT engine
                            ms2 = ws.tile([128, NT], F32, tag="ms2")
                            nc.vector.tensor_scalar(ms2[:], m_relu[:], 1e-6,
                                                    None, op0=ALU.max)
                            rk = ws.tile([128, NT], F32, tag="rk")
                            nc.vector.reciprocal(rk[:], ms2[:])
                            nc.vector.tensor_scalar(kscale[:], rk[:], -256.0,
                                                    None, op0=ALU.mult)
                            kpos = ws.tile([128, NT], F32, tag="kpos")
                            nc.vector.tensor_scalar(kpos[:], rk[:], 256.0,
                                                    None, op0=ALU.mult)
                            nc.vector.tensor_tensor(kbias[:], kpos[:],
                                                    m_relu[:], op=ALU.mult)
                            nc.vector.tensor_scalar(kbias[:], kbias[:], 1.0,
                                                    None, op0=ALU.add)

                            nc.vector.memset(wb_all[:], 0.0)
                            nc.vector.memset(wf_all[:], 0.0)
                            wbv = wb_all[:].rearrange("p (t k) -> p t k", k=32)
                            wfv = wf_all[:].rearrange("p (t k) -> p t k", k=32)
                            tv = trgb[:].rearrange("p (t k) -> p t k", k=3)
                            wgv = wgt[:].rearrange("p (t o) -> p t o", o=1)
                            for c in range(3):
                                nc.vector.tensor_tensor(
                                    wbv[:, :, c:c + 1], wgv, tv[:, :, c:c + 1],
                                    op=ALU.mult)
                                nc.vector.tensor_copy(wfv[:, :, c:c + 1],
                                                      tv[:, :, c:c + 1])
                            nc.vector.tensor_copy(wbv[:, :, 3:4], wgv)
                            nc.vector.memset(wfv[:, :, 3:4], 1.0)

                    if "C" in phases:
                        # --- colmin collapse: per-window transposes ran in
                        # pass A; transpose m2loc, DMA out, AllReduce(min) --
                        with (
                            tc.tile_pool(name="c_ps2", bufs=1,
                                         space="PSUM") as cps2,
                            tc.tile_pool(name="c_sb", bufs=1) as csb,
                        ):
                            pst2 = cps2.tile([nct, 128], F32, tag="pst2")
                            nc.tensor.transpose(pst2[:], m2loc[:], eye[:])
                            m2t = csb.tile([nct, 128], F32, tag="m2t")
                            nc.vector.tensor_copy(m2t[:], pst2[:])
                            nc.sync.dma_start(m2_in[:, :], m2t[:])
                        if "NOAR" in phases or NCORES == 1:
                            nc.sync.dma_start(
                                m2_out[0, :].rearrange("(p q) -> p q", p=nct),
                                m2_in[:, :])
                        else:
                            nc.gpsimd.collective_compute(
                                "AllReduce", ALU.min, replica_groups=rg,
                                ins=[m2_in.opt()], outs=[m2_out.opt()])

                if "FIN" in phases:
                    # ---- BCE term early: relu(p) - p*t + softplus(-|p|) ----
                    with tc.tile_pool(name="finE", bufs=1) as fe:
                        predf = fe.tile([128, lpb], F32, tag="predf",
                                        name="predf")
                        nc.sync.dma_start(
                            predf[:],
                            predd[0, :].rearrange("(p q) -> p q", p=128))
                        ktgt = fe.tile([128, lpb], F32, tag="ktgt", name="ktgt")
                        nc.sync.dma_start(
                            ktgt[:],
                            ktgtd[0, :].rearrange("(p q) -> p q", p=128))
                        bce = fe.tile([128, lpb], F32, tag="bce")
                        nc.scalar.activation(bce[:], predf[:], ACTF.Relu)
                        pt = fe.tile([128, lpb], F32, tag="pt")
                        nc.vector.tensor_tensor(pt[:], predf[:], ktgt[:],
                                                op=ALU.mult)
                        nc.vector.tensor_tensor(bce[:], bce[:], pt[:],
                                                op=ALU.subtract)
                        ap_ = fe.tile([128, lpb], F32, tag="ap_")
                        nc.scalar.activation(ap_[:], predf[:], ACTF.Abs)
                        en = fe.tile([128, lpb], F32, tag="en")
                        nc.scalar.activation(en[:], ap_[:], ACTF.Exp,
                                             scale=-1.0)
                        sp = fe.tile([128, lpb], F32, tag="sp")
                        nc.scalar.activation(sp[:], en[:], ACTF.Ln, bias=1.0)
                        nc.vector.tensor_tensor(bce[:], bce[:], sp[:],
                                                op=ALU.add)
                        nc.vector.tensor_reduce(rowsbce[:], bce[:], axis=AX.X,
                                                op=ALU.add)

                if "B" in phases:
                    # --- Pass B on the persisted d2: eb vs row min, ef vs
                    # global colmin.  All eb chunks run first (hiding the
                    # colmin AllReduce); each ef chunk's nd partials
                    # AllReduce(add) immediately so collectives overlap the
                    # next chunk's compute. ------------------------------
                    with (
                        tc.tile_pool(name="b_m2w", bufs=1) as bm2w,
                        tc.tile_pool(name="b_m2b", bufs=2) as bm2b,
                        tc.tile_pool(name="b_e", bufs=3) as bep,
                        tc.tile_pool(name="b_nd", bufs=1) as bnd,
                        tc.tile_pool(name="b_acc", bufs=2, space="PSUM") as baccp,
                    ):

                        def chunk_eb(ch):
                            csl = slice(ch * lch, (ch + 1) * lch)
                            acc = baccp.tile([128, 512], F32, tag="accb")
                            for t in range(NT):
                                eb = bep.tile([128, lch], BF16, tag="e")
                                if t < 7:
                                    # ACT-engine hat indicator (engine
                                    # balance: ACT ~= DVE in pass B)
                                    nc.scalar.activation(
                                        eb[:], d2p[t][:, csl], ACTF.Relu,
                                        bias=kbias[:, t:t + 1],
                                        scale=kscale[:, t:t + 1])
                                else:
                                    nc.vector.tensor_scalar(
                                        eb[:], d2p[t][:, csl],
                                        m_relu[:, t:t + 1],
                                        None, op0=ALU.is_equal)
                                for q in range(nq):
                                    nc.tensor.matmul(
                                        acc[32 * q:32 * q + 32, :],
                                        lhsT=wb_all[:, t * 32:(t + 1) * 32],
                                        rhs=eb[:, q * 512:(q + 1) * 512],
                                        start=(t == 0), stop=(t == NT - 1),
                                        tile_position=(0, 32 * q))
                            accs = bnd.tile([128, 512], F32, tag="ndb")
                            nc.scalar.copy(accs[:], acc[:])
                            for q in range(nq):
                                nc.sync.dma_start(
                                    nd_ins[ch][0:4, q * 512:(q + 1) * 512],
                                    accs[32 * q:32 * q + 4, :])

                        def bcast(ch):
                            # colmin chunk broadcast: issued on gpsimd BEFORE
                            # earlier chunks' AllReduces so the strict FIFO
                            # never stalls an ef compare behind a collective
                            csl = slice(ch * lch, (ch + 1) * lch)
                            m2w = bm2w.tile([1, lch], F32, tag="m2w")
                            nc.sync.dma_start(m2w[:], m2_out[:, csl])
                            m2wb = bm2w.tile([1, lch], BF16, tag="m2wb")
                            nc.scalar.copy(m2wb[:], m2w[:])
                            m2sl = bm2b.tile([128, lch], BF16, tag="m2b")
                            nc.gpsimd.partition_broadcast(m2sl[:], m2wb[:])
                            return m2sl

                        def chunk_ef(ch, m2sl):
                            csl = slice(ch * lch, (ch + 1) * lch)
                            acc = baccp.tile([128, 512], F32, tag="accf")
                            for t in range(NT):
                                ef = bep.tile([128, lch], BF16, tag="e")
                                nc.vector.tensor_tensor(
                                    ef[:], d2p[t][:, csl], m2sl[:],
                                    op=ALU.is_le)
                                for q in range(nq):
                                    nc.tensor.matmul(
                                        acc[32 * q:32 * q + 32, :],
                                        lhsT=wf_all[:, t * 32:(t + 1) * 32],
                                        rhs=ef[:, q * 512:(q + 1) * 512],
                                        start=(t == 0), stop=(t == NT - 1),
                                        tile_position=(0, 32 * q))
                            accs = bnd.tile([128, 512], F32, tag="ndf")
                            nc.scalar.copy(accs[:], acc[:])
                            for q in range(nq):
                                nc.sync.dma_start(
                                    nd_ins[ch][4:8, q * 512:(q + 1) * 512],
                                    accs[32 * q:32 * q + 4, :])

                        def reduce_chunk(ch):
                            if "NOAR" in phases:
                                nc.sync.dma_start(nd_outs[ch][:, :],
                                                  nd_ins[ch][:, :])
                            elif NCORES > 1:
                                nc.gpsimd.collective_compute(
                                    "AllReduce", ALU.add, replica_groups=rg,
                                    ins=[nd_ins[ch].opt()],
                                    outs=[nd_outs[ch].opt()])
                            else:
                                nc.sync.dma_start(nd_outs[ch][:, :],
                                                  nd_ins[ch][:, :])

                        # gpsimd FIFO order: b0 b1 AR0 b2 AR1 b3 AR2 AR3 —
                        # every broadcast precedes the AllReduces it must
                        # not wait on; eb chunks 0/1 cover the colmin AR
                        chunk_eb(0)
                        chunk_eb(1)
                        m2s0 = bcast(0)
                        chunk_ef(0, m2s0)
                        m2s1 = bcast(1)
                        reduce_chunk(0)
                        chunk_eb(2)
                        chunk_ef(1, m2s1)
                        m2s2 = bcast(2)
                        reduce_chunk(1)
                        chunk_eb(3)
                        chunk_ef(2, m2s2)
                        m2s3 = bcast(3)
                        reduce_chunk(2)
                        chunk_ef(3, m2s3)
                        reduce_chunk(3)

                if "FIN" in phases:
                    # ---- finalize, chunked by nd AllReduce chunk ----------
                    prow = lch // lpf  # plane partitions per nd chunk
                    with (
                        tc.tile_pool(name="fin", bufs=1) as fp,
                        tc.tile_pool(name="fin_ps", bufs=1, space="PSUM") as fps,
                    ):
                        accp = fp.tile([prow, NDCH], F32, tag="accp")
                        for ch in range(NDCH):
                            j0 = ch * lch

                            def plane(dram_row, tg):
                                tl = fp.tile([prow, lpf], F32, tag=tg,
                                             name=f"{tg}_{ch}")
                                nc.sync.dma_start(
                                    tl[:], dram_row.rearrange("(p q) -> p q",
                                                              p=prow))
                                return tl

                            def plane_nd(k, tg):
                                tl = fp.tile([prow, lpf], F32, tag=tg,
                                             name=f"{tg}_{ch}")
                                nc.sync.dma_start(
                                    tl[:], nd_outs[ch][k, :].rearrange(
                                        "(p q) -> p q", p=prow))
                                return tl

                            rgbp = [plane(rgbpd[k, j0:j0 + lch], f"rgb{k}")
                                    for k in range(3)]
                            keepf = plane(keepd[0, j0:j0 + lch], "keepf")
                            nd = [plane_nd(k, f"nd{k}") for k in range(8)]

                            num, den = nd[0:3], nd[3]
                            sf, cntf = nd[4:7], nd[7]

                            _cnt = [0]

                            def newt():
                                _cnt[0] += 1
                                return fp.tile([prow, lpf], F32,
                                               tag=f"fin{_cnt[0]}",
                                               name=f"fin{_cnt[0]}_{ch}")

                            dsafe = newt()
                            nc.vector.tensor_scalar(dsafe[:], den[:], 0.0,
                                                    None, op0=ALU.is_equal)
                            nc.vector.tensor_tensor(dsafe[:], dsafe[:], den[:],
                                                    op=ALU.add)
                            rden = newt()
                            nc.vector.reciprocal(rden[:], dsafe[:])
                            rcntf = newt()
                            nc.vector.reciprocal(rcntf[:], cntf[:])

                            mden = fp.tile([prow, lpf], mybir.dt.int32,
                                           tag="mden", name=f"mden_{ch}")
                            nc.vector.tensor_scalar(mden[:], den[:], 0.0, None,
                                                    op0=ALU.not_equal)

                            acc = newt()
                            nc.vector.memset(acc[:], 0.0)
                            for c in range(3):
                                rec = newt()
                                nc.vector.tensor_tensor(rec[:], sf[c][:],
                                                        rcntf[:], op=ALU.mult)
                                tmp = newt()
                                nc.vector.tensor_tensor(tmp[:], num[c][:],
                                                        rden[:], op=ALU.mult)
                                nc.vector.copy_predicated(rec[:], mden[:],
                                                          tmp[:])
                                diff = newt()
                                nc.vector.tensor_tensor(diff[:], rgbp[c][:],
                                                        rec[:],
                                                        op=ALU.subtract)
                                ad = newt()
                                nc.scalar.activation(ad[:], diff[:], ACTF.Abs)
                                nc.vector.tensor_tensor(acc[:], acc[:], ad[:],
                                                        op=ALU.add)
                            nc.vector.tensor_tensor(acc[:], acc[:], keepf[:],
                                                    op=ALU.mult)
                            nc.vector.tensor_reduce(accp[:, ch:ch + 1], acc[:],
                                                    axis=AX.X, op=ALU.add)

                        onescol = fp.tile([128, 1], F32, tag="onescol")
                        nc.vector.memset(onescol[:], 1.0)
                        ps_a = fps.tile([1, 1], F32, tag="ps_a")
                        nc.tensor.matmul(ps_a[:], lhsT=onescol[:],
                                         rhs=rowsbce[:], start=True, stop=True)
                        ps_b = fps.tile([1, NDCH], F32, tag="ps_b")
                        nc.tensor.matmul(ps_b[:], lhsT=onescol[0:prow, :],
                                         rhs=accp[:], start=True, stop=True)
                        chsb = fp.tile([1, 2], F32, tag="chsb")
                        nc.sync.dma_start(chsb[:], chaind[:, :])
                        nc.vector.tensor_scalar(chsb[:], chsb[:], 0.0, None,
                                                op0=ALU.mult)
                        outsb = fp.tile([1, 2], F32, tag="outsb")
                        nc.scalar.copy(outsb[:, 0:1], ps_a[:])
                        nc.vector.tensor_reduce(outsb[:, 1:2], ps_b[:],
                                                axis=AX.X, op=ALU.add)
                        nc.vector.tensor_tensor(outsb[:], outsb[:], chsb[:],
                                                op=ALU.add)
                        nc.sync.dma_start(outd[:, :], outsb[:])

    nc.compile()
    return nc


def _host_prep(pred_F, cand_xyz, cand_rgb, tgt_xyz, tgt_rgb, keep_target,
               points_num):
    bf16 = mybir.dt.np(BF16)
    nsh = N // NCORES
    npad = NT * 128
    pred = np.ascontiguousarray(np.asarray(pred_F, np.float32))
    cxyz = np.ascontiguousarray(np.asarray(cand_xyz, np.float32))
    crgb = np.ascontiguousarray(np.asarray(cand_rgb, np.float32))
    txyz = np.ascontiguousarray(np.asarray(tgt_xyz, np.float32))
    trgb_np = np.ascontiguousarray(np.asarray(tgt_rgb, np.float32))
    ktgt = np.asarray(keep_target).astype(np.float32)

    # keep mask (exact reference semantics, f32)
    Lfull = pred.shape[0]
    p8 = pred.reshape(-1, 8)
    rows = np.arange(p8.shape[0])
    ilm = np.zeros(p8.shape, dtype=bool)
    ilm[rows, np.argmax(p8, axis=1)] = True
    ilm = ilm.reshape(-1)
    k = Lfull - int(points_num)
    vals = np.where(ilm, np.inf, pred)
    thr = np.sort(vals)[k - 1]
    keep = (pred > thr) | ilm

    # compact kept candidates to LK columns (pad: b2=BIG never wins a min,
    # keepf=0 never reaches the loss)
    kidx = np.nonzero(keep)[0]
    nk = kidx.size
    assert nk <= LK, f"kept={nk} > {LK}"
    cxyz_k = np.zeros((LK, 3), np.float32)
    cxyz_k[:nk] = cxyz[kidx]
    b2 = np.sum(cxyz * cxyz, axis=1, dtype=np.float32).astype(np.float32)
    b2k = np.full((LK,), BIG, np.float32)
    b2k[:nk] = b2[kidx]
    ones = np.ones(LK, np.float32)
    c5_np = np.stack([cxyz_k[:, 0], cxyz_k[:, 1], cxyz_k[:, 2], ones, b2k])
    # group by tile-position: [5, g, jc*512+u] <- [5, jc*2048 + g*512 + u]
    c5v = c5_np.reshape(5, LK // AT_W, NGA, 512)
    c5g_np = np.ascontiguousarray(
        c5v.transpose(0, 2, 1, 3).reshape(5, LK).astype(bf16))

    rgbp = np.zeros((3, LK), np.float32)
    rgbp[:, :nk] = (crgb[kidx] * np.float32(255.0)).T
    rgbp = np.ascontiguousarray(rgbp)
    keepf = np.zeros((1, LK), np.float32)
    keepf[0, :nk] = 1.0

    a2 = np.sum(txyz * txyz, axis=1, dtype=np.float32).astype(np.float32)

    t5_cores, trgb_cores = [], []
    for c in range(NCORES):
        sl = slice(c * nsh, (c + 1) * nsh)
        t5 = np.zeros((5, npad), np.float32)
        t5[3, :] = BIG     # pad rows: s = 1e30 everywhere
        t5[4, :] = 1.0
        t5[0, :nsh] = -2.0 * txyz[sl, 0]
        t5[1, :nsh] = -2.0 * txyz[sl, 1]
        t5[2, :nsh] = -2.0 * txyz[sl, 2]
        t5[3, :nsh] = a2[sl]
        tr = np.zeros((npad, 3), np.float32)
        tr[:nsh] = trgb_np[sl]
        # [p, t*3+c] layout: target i_local = t*128 + p
        trc = tr.reshape(NT, 128, 3).transpose(1, 0, 2).reshape(128, NT * 3)
        t5_cores.append(np.ascontiguousarray(t5.astype(bf16)))
        trgb_cores.append(np.ascontiguousarray(trc))

    eye = np.eye(128, dtype=np.float32)

    common = dict(c5g=c5g_np, rgbp=rgbp,
                  keepf=keepf, predf=pred.reshape(1, Lfull),
                  ktgt=ktgt.reshape(1, Lfull), eye128=eye,
                  chain=np.zeros((1, 2), np.float32))
    in_maps = [dict(common, t5=t5_cores[c], trgb=trgb_cores[c])
               for c in range(NCORES)]
    return in_maps


_CACHE = {}


def kernel(pred_F, cand_xyz, cand_rgb, tgt_xyz, tgt_rgb, keep_target,
           points_num=8192, **_ignored):
    in_maps = _host_prep(pred_F, cand_xyz, cand_rgb, tgt_xyz, tgt_rgb,
                         keep_target, points_num)
    if "nc" not in _CACHE:
        _CACHE["nc"] = _build_nc()
    res = run_bass_kernel_spmd(_CACHE["nc"], in_maps,
                               core_ids=list(range(NCORES)))
    return np.asarray(res.results[0]["out"], np.float32).reshape(2)


if __name__ == "__main__":
    import reference as R
    inputs = R.setup_inputs()
    inputs = {kk: np.asarray(vv) if not np.isscalar(vv) else vv
              for kk, vv in inputs.items()}
    out = kernel(**inputs)
    print("kernel out:", out)
